# revision 16
# baseline (speedup 1.0000x reference)
"""Trainium2 Bass kernel for bidirectional ActionLSTM.

Full inputs in, full output out. Internally: data-parallel over batch
(8 NeuronCores x 256 batch rows), LSTM weights replicated.

Device program (per core, transposed layout: hidden on partitions,
batch on the free dim):
  - fc_in is folded into the LSTM input weights on the host:
        W_x = w_ih @ fc_in_w  [512, 68],  bias = w_ih@fc_in_b + b_ih + b_hh
    and the bias is folded in as an extra all-ones input row (K=69).
  - Gate order permuted to [i, f, o, g]. tanh is computed via the
    identity tanh(z) = 2*sigmoid(2z) - 1, with the 2z folded into the
    g-gate weight rows, so ONE sigmoid activation op covers all gates.
  - h is stored as h/2 ("h_half"); the 2x is folded into the recurrent
    and pooling weights. tanh(c) likewise becomes sigmoid(2c) via the
    activation's free scale.
  - Mean-pool over time + fc_out are folded into a per-step accumulating
    matmul into PSUM: pacc += (2/T * Wo_dir) @ h_half_t.
Per (step, dir): 9 matmuls (4 x-side K=69 + 4 recurrent K=128 + 1 pool),
1 big sigmoid [128,1024], 1 sigmoid(2c) [128,256] on ACT, 3
scalar_tensor_tensor ops on DVE, 1 tensor_tensor on GpSimd.
"""

import os
import numpy as np
from contextlib import ExitStack

INPUT, HID, NCLS = 68, 128, 3
B, T = 2048, 128
NCORES = 8
BL = B // NCORES          # 256 batch rows per core
KX = INPUT + 1            # 69 (ones row folds bias in)
G4 = 4 * HID              # 512

# matmul operand dtype: "f32r" (full fp32 storage, fast tensor-engine mode)
# or "bf16"
MM_DTYPE = os.environ.get("LSTM_MM_DTYPE", "bf16")
USE_FILLER = os.environ.get("LSTM_FILLER", "1") == "1"
N_FILLER = int(os.environ.get("LSTM_N_FILLER", "3"))

_CACHE = {}


def _build_program():
    import concourse.bass as bass
    import concourse.tile as tile
    from concourse import bacc, mybir

    f32 = mybir.dt.float32
    AF = mybir.ActivationFunctionType
    OP = mybir.AluOpType
    use_bf16 = MM_DTYPE == "bf16"
    # dtype for tensors consumed by the tensor engine (x, weights, h):
    # bf16 runs the matmul at full rate (fp32r measured at 1/4 rate on HW);
    # gate accumulation stays fp32 in PSUM, s/c stay fp32 on DVE/ACT.
    sb_dt = mybir.dt.bfloat16 if use_bf16 else mybir.dt.float32r

    def R(ap):
        return ap

    nc = bacc.Bacc("TRN2", target_bir_lowering=False, debug=False,
                   num_devices=NCORES)

    xin = nc.dram_tensor("xin", [KX, T * BL], sb_dt, kind="ExternalInput").ap()
    wx = {}
    wu = {}
    for d in "fb":
        wx[d] = nc.dram_tensor(f"wx_{d}", [KX, G4], sb_dt,
                               kind="ExternalInput").ap()
        wu[d] = nc.dram_tensor(f"wu_{d}", [HID, G4], sb_dt,
                               kind="ExternalInput").ap()
    out = nc.dram_tensor("out", [2 * HID, BL], f32, kind="ExternalOutput").ap()

    with tile.TileContext(nc) as tc, ExitStack() as ctx:
        const = ctx.enter_context(tc.tile_pool(name="const", bufs=1))
        X = const.tile([KX, T * BL], sb_dt, tag="X")
        # split the big input DMA into chunks so it spreads across DMA
        # queues and so early timesteps unblock compute quickly; issue
        # from both ends since the bwd direction consumes t=T-1 first.
        NCHUNK = 16
        CW = T * BL // NCHUNK
        order = []
        for i in range(NCHUNK // 2):
            order += [NCHUNK - 1 - i, i]
        for ci in order:
            nc.sync.dma_start(X[:, ci * CW:(ci + 1) * CW],
                              xin[:, ci * CW:(ci + 1) * CW])

        WX = {}
        WU = {}
        for d in "fb":
            WX[d] = const.tile([KX, G4], sb_dt, tag=f"wx{d}", name=f"WX{d}")
            nc.sync.dma_start(WX[d][:], wx[d][:])
            WU[d] = const.tile([HID, G4], sb_dt, tag=f"wu{d}", name=f"WU{d}")
            nc.sync.dma_start(WU[d][:], wu[d][:])

        hpool = ctx.enter_context(tc.tile_pool(name="h", bufs=3))
        cpool = ctx.enter_context(tc.tile_pool(name="c", bufs=3))
        spool = ctx.enter_context(tc.tile_pool(name="s", bufs=2))
        scpool = ctx.enter_context(tc.tile_pool(name="sc", bufs=2))
        mpool = ctx.enter_context(tc.tile_pool(name="m1h", bufs=2))
        tpool = ctx.enter_context(tc.tile_pool(name="tt", bufs=2))
        gpsum = ctx.enter_context(tc.tile_pool(name="gates", bufs=1,
                                               space="PSUM"))

        h = {}
        c = {}
        hsum = {}
        gates = {}
        for d in "fb":
            h[d] = hpool.tile([HID, BL], sb_dt, tag=f"h{d}", name=f"h0{d}")
            nc.vector.memset(h[d][:].bitcast(f32) if not use_bf16 else h[d][:],
                             0.0)
            c[d] = cpool.tile([HID, BL], f32, tag=f"c{d}", name=f"c0{d}")
            nc.vector.memset(c[d][:], 0.0)
            hsum[d] = const.tile([HID, BL], f32, tag=f"hs{d}", name=f"hsum{d}")
            nc.vector.memset(hsum[d][:], 0.0)

        for t in range(T):
            # phase 0: x-side window matmuls for both dirs (independent of
            # h; keeping them at the PE queue head avoids blocking behind
            # the h-gated U matmuls)
            for d in ("f", "b"):
                if t % 2 == 0:
                    g = gpsum.tile([HID, 4, 2 * BL], f32, tag=f"g{d}",
                                   name=f"g_{d}_{t}")
                    gates[d] = g
                    if d == "f":
                        xw = X[:, t * BL:(t + 2) * BL]
                    else:
                        xw = X[:, (T - 2 - t) * BL:(T - t) * BL]
                    for gi in range(4):
                        nc.tensor.matmul(g[:, gi, :],
                                         R(WX[d][:, gi * HID:(gi + 1) * HID]),
                                         R(xw), start=True, stop=False,
                                         skip_group_check=True)
            # phase 1: recurrent matmuls + big sigmoid, per dir — the two
            # dirs' sigmoids sit adjacent in the ACT FIFO so ACT works dir b
            # while dir f's DVE chain runs (anti-phase staggering)
            sd = {}
            half = {}
            for d in ("f", "b"):
                g = gates[d]
                hf = t % 2 if d == "f" else 1 - (t % 2)
                half[d] = hf
                cs = hf * BL
                for gi in range(4):
                    nc.tensor.matmul(g[:, gi, cs:cs + BL],
                                     R(WU[d][:, gi * HID:(gi + 1) * HID]),
                                     R(h[d][:]), start=False, stop=(t % 2 == 1),
                                     skip_group_check=True)
                s = spool.tile([HID, 4, BL], f32, tag=f"s{d}", name=f"s{d}{t}")
                nc.scalar.activation(s[:], g[:, :, cs:cs + BL], AF.Sigmoid)
                sd[d] = s
            # phase 2: cell updates, stage-interleaved across dirs
            ttd = {}
            md = {}
            cn = {}
            scd = {}
            for d in ("f", "b"):
                s = sd[d]
                tt = tpool.tile([HID, BL], f32, tag=f"tt{d}", name=f"tt{d}{t}")
                nc.gpsimd.tensor_tensor(tt[:], s[:, 1, :], c[d][:], op=OP.mult)
                m1h = mpool.tile([HID, BL], f32, tag=f"m{d}", name=f"m{d}{t}")
                nc.vector.scalar_tensor_tensor(m1h[:], s[:, 3, :], 0.5,
                                               s[:, 0, :],
                                               op0=OP.subtract, op1=OP.mult)
                ttd[d] = tt
                md[d] = m1h
            for d in ("f", "b"):
                c_new = cpool.tile([HID, BL], f32, tag=f"c{d}", name=f"c{d}{t}")
                nc.vector.tensor_tensor(c_new[:], md[d][:], ttd[d][:],
                                        op=OP.add)
                sc = scpool.tile([HID, BL], f32, tag=f"sc{d}", name=f"sc{d}{t}")
                nc.scalar.activation(sc[:], c_new[:], AF.Sigmoid, scale=4.0)
                cn[d] = c_new
                scd[d] = sc
            for d in ("f", "b"):
                h_new = hpool.tile([HID, BL], sb_dt, tag=f"h{d}",
                                   name=f"h{d}{t}")
                nc.vector.scalar_tensor_tensor(h_new[:], scd[d][:], 0.5,
                                               sd[d][:, 2, :],
                                               op0=OP.subtract, op1=OP.mult)
                # pooling accumulate on the otherwise-idle GpSimd (off the
                # critical path, so the slow Q7 path is fine)
                nc.gpsimd.tensor_tensor(hsum[d][:], hsum[d][:], h_new[:],
                                        op=OP.add)
                h[d] = h_new
                c[d] = cn[d]

        nc.sync.dma_start(out[0:HID, :], hsum["f"][:])
        nc.sync.dma_start(out[HID:2 * HID, :], hsum["b"][:])

    nc.compile()
    return nc


def _prep_weights(w_ih, w_hh, b_ih, b_hh, fc_in_w, fc_in_b):
    Wx = w_ih.astype(np.float64) @ fc_in_w.astype(np.float64)   # [512, 68]
    bias = w_ih.astype(np.float64) @ fc_in_b.astype(np.float64) \
        + b_ih.astype(np.float64) + b_hh.astype(np.float64)
    perm = np.concatenate([np.arange(0, 128), np.arange(128, 256),
                           np.arange(384, 512), np.arange(256, 384)])
    Wx = Wx[perm]
    U = w_hh.astype(np.float64)[perm]
    bias = bias[perm]
    srow = np.ones((512, 1), np.float64)
    srow[384:] = 2.0
    Wx_aug = np.concatenate([Wx, bias[:, None]], axis=1)        # [512, 69]
    lhsT_x = np.ascontiguousarray((srow * Wx_aug).T)            # [69, 512]
    lhsT_U = np.ascontiguousarray((srow * U * 2.0).T)           # [128, 512]
    return lhsT_x, lhsT_U


def kernel(x, fc_in_w, fc_in_b, w_ih_f, w_hh_f, b_ih_f, b_hh_f,
           w_ih_b, w_hh_b, b_ih_b, b_hh_b, fc_out_w, fc_out_b,
           _want_trace=False):
    from concourse import bass_utils

    np_dt = np.float32
    if MM_DTYPE == "bf16":
        import ml_dtypes
        np_dt = ml_dtypes.bfloat16

    if "nc" not in _CACHE:
        _CACHE["nc"] = _build_program()
    nc = _CACHE["nc"]

    lx_f, lU_f = _prep_weights(w_ih_f, w_hh_f, b_ih_f, b_hh_f,
                               fc_in_w, fc_in_b)
    lx_b, lU_b = _prep_weights(w_ih_b, w_hh_b, b_ih_b, b_hh_b,
                               fc_in_w, fc_in_b)
    shared = {
        "wx_f": lx_f.astype(np_dt), "wu_f": lU_f.astype(np_dt),
        "wx_b": lx_b.astype(np_dt), "wu_b": lU_b.astype(np_dt),
    }
    wo_f = fc_out_w[:, :HID].astype(np.float64)   # [3, 128]
    wo_b = fc_out_w[:, HID:].astype(np.float64)
    in_maps = []
    for cidx in range(NCORES):
        xs = x[cidx * BL:(cidx + 1) * BL]                    # [BL, T, 68]
        xT = np.ascontiguousarray(xs.transpose(2, 1, 0))     # [68, T, BL]
        x_aug = np.concatenate(
            [xT, np.ones((1, T, BL), np.float32)], axis=0)   # [69, T, BL]
        in_maps.append({"xin": x_aug.reshape(KX, T * BL).astype(np_dt),
                        **shared})

    res = bass_utils.run_bass_kernel_spmd(
        nc, in_maps, core_ids=list(range(NCORES)), trace=_want_trace)
    outs = []
    for cidx in range(NCORES):
        o = res.results[cidx]["out"].astype(np.float64)       # [2H, BL]
        pool = (2.0 / T) * (wo_f @ o[0:HID] + wo_b @ o[HID:])  # [3, BL]
        out_core = pool.T + fc_out_b                          # [BL, 3]
        outs.append(out_core)
    full = np.concatenate(outs, axis=0).astype(np.float32)
    if _want_trace:
        _CACHE["last_result"] = res
    return full


# revision 17
# speedup vs baseline: 1.2499x; 1.2499x over previous
"""Trainium2 Bass kernel for bidirectional ActionLSTM.

Full inputs in, full output out. Internally: data-parallel over batch
(8 NeuronCores x 256 batch rows), LSTM weights replicated.

Device program (per core, transposed layout: hidden on partitions,
batch on the free dim):
  - fc_in is folded into the LSTM input weights on the host:
        W_x = w_ih @ fc_in_w  [512, 68],  bias = w_ih@fc_in_b + b_ih + b_hh
    and the bias is folded in as an extra all-ones input row (K=69).
  - Gate order permuted to [i, f, o, g]. tanh is computed via the
    identity tanh(z) = 2*sigmoid(2z) - 1, with the 2z folded into the
    g-gate weight rows, so ONE sigmoid activation op covers all gates.
  - h is stored as h/2 ("h_half"); the 2x is folded into the recurrent
    and pooling weights. tanh(c) likewise becomes sigmoid(2c) via the
    activation's free scale.
  - Mean-pool over time + fc_out are folded into a per-step accumulating
    matmul into PSUM: pacc += (2/T * Wo_dir) @ h_half_t.
Per (step, dir): 9 matmuls (4 x-side K=69 + 4 recurrent K=128 + 1 pool),
1 big sigmoid [128,1024], 1 sigmoid(2c) [128,256] on ACT, 3
scalar_tensor_tensor ops on DVE, 1 tensor_tensor on GpSimd.
"""

import os
import numpy as np
from contextlib import ExitStack

INPUT, HID, NCLS = 68, 128, 3
B, T = 2048, 128
NCORES = 8
BL = B // NCORES          # 256 batch rows per core
KX = INPUT + 1            # 69 (ones row folds bias in)
G4 = 4 * HID              # 512

# matmul operand dtype: "f32r" (full fp32 storage, fast tensor-engine mode)
# or "bf16"
MM_DTYPE = os.environ.get("LSTM_MM_DTYPE", "bf16")
USE_FILLER = os.environ.get("LSTM_FILLER", "1") == "1"
N_FILLER = int(os.environ.get("LSTM_N_FILLER", "3"))
CELL_BF16 = os.environ.get("LSTM_CELL_BF16", "1") == "1"

_CACHE = {}


def _build_program():
    import concourse.bass as bass
    import concourse.tile as tile
    from concourse import bacc, mybir

    f32 = mybir.dt.float32
    AF = mybir.ActivationFunctionType
    OP = mybir.AluOpType
    use_bf16 = MM_DTYPE == "bf16"
    # dtype for tensors consumed by the tensor engine (x, weights, h):
    # bf16 runs the matmul at full rate (fp32r measured at 1/4 rate on HW);
    # gate accumulation stays fp32 in PSUM, s/c stay fp32 on DVE/ACT.
    sb_dt = mybir.dt.bfloat16 if use_bf16 else mybir.dt.float32r

    cell_dt = mybir.dt.bfloat16 if CELL_BF16 else f32

    def R(ap):
        return ap

    nc = bacc.Bacc("TRN2", target_bir_lowering=False, debug=False,
                   num_devices=NCORES)

    xin = nc.dram_tensor("xin", [KX, T * BL], sb_dt, kind="ExternalInput").ap()
    wx = {}
    wu = {}
    for d in "fb":
        wx[d] = nc.dram_tensor(f"wx_{d}", [KX, G4], sb_dt,
                               kind="ExternalInput").ap()
        wu[d] = nc.dram_tensor(f"wu_{d}", [HID, G4], sb_dt,
                               kind="ExternalInput").ap()
    out = nc.dram_tensor("out", [2 * HID, BL], f32, kind="ExternalOutput").ap()

    with tile.TileContext(nc) as tc, ExitStack() as ctx:
        const = ctx.enter_context(tc.tile_pool(name="const", bufs=1))
        X = const.tile([KX, T * BL], sb_dt, tag="X")
        # split the big input DMA into chunks so it spreads across DMA
        # queues and so early timesteps unblock compute quickly; issue
        # from both ends since the bwd direction consumes t=T-1 first.
        NCHUNK = 16
        CW = T * BL // NCHUNK
        order = []
        for i in range(NCHUNK // 2):
            order += [NCHUNK - 1 - i, i]
        for ci in order:
            nc.sync.dma_start(X[:, ci * CW:(ci + 1) * CW],
                              xin[:, ci * CW:(ci + 1) * CW])

        WX = {}
        WU = {}
        for d in "fb":
            WX[d] = const.tile([KX, G4], sb_dt, tag=f"wx{d}", name=f"WX{d}")
            nc.sync.dma_start(WX[d][:], wx[d][:])
            WU[d] = const.tile([HID, G4], sb_dt, tag=f"wu{d}", name=f"WU{d}")
            nc.sync.dma_start(WU[d][:], wu[d][:])

        hpool = ctx.enter_context(tc.tile_pool(name="h", bufs=3))
        cpool = ctx.enter_context(tc.tile_pool(name="c", bufs=3))
        spool = ctx.enter_context(tc.tile_pool(name="s", bufs=2))
        scpool = ctx.enter_context(tc.tile_pool(name="sc", bufs=2))
        mpool = ctx.enter_context(tc.tile_pool(name="m1h", bufs=2))
        tpool = ctx.enter_context(tc.tile_pool(name="tt", bufs=2))
        gpsum = ctx.enter_context(tc.tile_pool(name="gates", bufs=1,
                                               space="PSUM"))

        h = {}
        c = {}
        hsum = {}
        gates = {}
        for d in "fb":
            h[d] = hpool.tile([HID, BL], sb_dt, tag=f"h{d}", name=f"h0{d}")
            nc.vector.memset(h[d][:].bitcast(f32) if not use_bf16 else h[d][:],
                             0.0)
            c[d] = cpool.tile([HID, BL], cell_dt, tag=f"c{d}", name=f"c0{d}")
            nc.vector.memset(c[d][:], 0.0)
            hsum[d] = const.tile([HID, BL], f32, tag=f"hs{d}", name=f"hsum{d}")
            nc.vector.memset(hsum[d][:], 0.0)

        for t in range(T):
            # phase 0: x-side window matmuls for both dirs (independent of
            # h; keeping them at the PE queue head avoids blocking behind
            # the h-gated U matmuls)
            for d in ("f", "b"):
                if t % 2 == 0:
                    g = gpsum.tile([HID, 4, 2 * BL], f32, tag=f"g{d}",
                                   name=f"g_{d}_{t}")
                    gates[d] = g
                    if d == "f":
                        xw = X[:, t * BL:(t + 2) * BL]
                    else:
                        xw = X[:, (T - 2 - t) * BL:(T - t) * BL]
                    for gi in range(4):
                        nc.tensor.matmul(g[:, gi, :],
                                         R(WX[d][:, gi * HID:(gi + 1) * HID]),
                                         R(xw), start=True, stop=False,
                                         skip_group_check=True)
            # phase 1: recurrent matmuls + big sigmoid, per dir — the two
            # dirs' sigmoids sit adjacent in the ACT FIFO so ACT works dir b
            # while dir f's DVE chain runs (anti-phase staggering)
            sd = {}
            half = {}
            for d in ("f", "b"):
                g = gates[d]
                hf = t % 2 if d == "f" else 1 - (t % 2)
                half[d] = hf
                cs = hf * BL
                for gi in range(4):
                    nc.tensor.matmul(g[:, gi, cs:cs + BL],
                                     R(WU[d][:, gi * HID:(gi + 1) * HID]),
                                     R(h[d][:]), start=False, stop=(t % 2 == 1),
                                     skip_group_check=True)
                s = spool.tile([HID, 4, BL], cell_dt, tag=f"s{d}", name=f"s{d}{t}")
                nc.scalar.activation(s[:], g[:, :, cs:cs + BL], AF.Sigmoid)
                sd[d] = s
            # phase 2: cell updates, stage-interleaved across dirs
            ttd = {}
            md = {}
            cn = {}
            scd = {}
            for d in ("f", "b"):
                s = sd[d]
                tt = tpool.tile([HID, BL], cell_dt, tag=f"tt{d}", name=f"tt{d}{t}")
                nc.vector.tensor_tensor(tt[:], s[:, 1, :], c[d][:], op=OP.mult)
                m1h = mpool.tile([HID, BL], cell_dt, tag=f"m{d}", name=f"m{d}{t}")
                nc.vector.scalar_tensor_tensor(m1h[:], s[:, 3, :], 0.5,
                                               s[:, 0, :],
                                               op0=OP.subtract, op1=OP.mult)
                ttd[d] = tt
                md[d] = m1h
            for d in ("f", "b"):
                c_new = cpool.tile([HID, BL], cell_dt, tag=f"c{d}", name=f"c{d}{t}")
                nc.vector.tensor_tensor(c_new[:], md[d][:], ttd[d][:],
                                        op=OP.add)
                sc = scpool.tile([HID, BL], cell_dt, tag=f"sc{d}", name=f"sc{d}{t}")
                nc.scalar.activation(sc[:], c_new[:], AF.Sigmoid, scale=4.0)
                cn[d] = c_new
                scd[d] = sc
            for d in ("f", "b"):
                h_new = hpool.tile([HID, BL], sb_dt, tag=f"h{d}",
                                   name=f"h{d}{t}")
                nc.vector.scalar_tensor_tensor(h_new[:], scd[d][:], 0.5,
                                               sd[d][:, 2, :],
                                               op0=OP.subtract, op1=OP.mult)
                # pooling accumulate on the otherwise-idle GpSimd (off the
                # critical path, so the slow Q7 path is fine)
                nc.gpsimd.tensor_tensor(hsum[d][:], hsum[d][:], h_new[:],
                                        op=OP.add)
                h[d] = h_new
                c[d] = cn[d]

        nc.sync.dma_start(out[0:HID, :], hsum["f"][:])
        nc.sync.dma_start(out[HID:2 * HID, :], hsum["b"][:])

    nc.compile()
    return nc


def _prep_weights(w_ih, w_hh, b_ih, b_hh, fc_in_w, fc_in_b):
    Wx = w_ih.astype(np.float64) @ fc_in_w.astype(np.float64)   # [512, 68]
    bias = w_ih.astype(np.float64) @ fc_in_b.astype(np.float64) \
        + b_ih.astype(np.float64) + b_hh.astype(np.float64)
    perm = np.concatenate([np.arange(0, 128), np.arange(128, 256),
                           np.arange(384, 512), np.arange(256, 384)])
    Wx = Wx[perm]
    U = w_hh.astype(np.float64)[perm]
    bias = bias[perm]
    srow = np.ones((512, 1), np.float64)
    srow[384:] = 2.0
    Wx_aug = np.concatenate([Wx, bias[:, None]], axis=1)        # [512, 69]
    lhsT_x = np.ascontiguousarray((srow * Wx_aug).T)            # [69, 512]
    lhsT_U = np.ascontiguousarray((srow * U * 2.0).T)           # [128, 512]
    return lhsT_x, lhsT_U


def kernel(x, fc_in_w, fc_in_b, w_ih_f, w_hh_f, b_ih_f, b_hh_f,
           w_ih_b, w_hh_b, b_ih_b, b_hh_b, fc_out_w, fc_out_b,
           _want_trace=False):
    from concourse import bass_utils

    np_dt = np.float32
    if MM_DTYPE == "bf16":
        import ml_dtypes
        np_dt = ml_dtypes.bfloat16

    if "nc" not in _CACHE:
        _CACHE["nc"] = _build_program()
    nc = _CACHE["nc"]

    lx_f, lU_f = _prep_weights(w_ih_f, w_hh_f, b_ih_f, b_hh_f,
                               fc_in_w, fc_in_b)
    lx_b, lU_b = _prep_weights(w_ih_b, w_hh_b, b_ih_b, b_hh_b,
                               fc_in_w, fc_in_b)
    shared = {
        "wx_f": lx_f.astype(np_dt), "wu_f": lU_f.astype(np_dt),
        "wx_b": lx_b.astype(np_dt), "wu_b": lU_b.astype(np_dt),
    }
    wo_f = fc_out_w[:, :HID].astype(np.float64)   # [3, 128]
    wo_b = fc_out_w[:, HID:].astype(np.float64)
    in_maps = []
    for cidx in range(NCORES):
        xs = x[cidx * BL:(cidx + 1) * BL]                    # [BL, T, 68]
        xT = np.ascontiguousarray(xs.transpose(2, 1, 0))     # [68, T, BL]
        x_aug = np.concatenate(
            [xT, np.ones((1, T, BL), np.float32)], axis=0)   # [69, T, BL]
        in_maps.append({"xin": x_aug.reshape(KX, T * BL).astype(np_dt),
                        **shared})

    res = bass_utils.run_bass_kernel_spmd(
        nc, in_maps, core_ids=list(range(NCORES)), trace=_want_trace)
    outs = []
    for cidx in range(NCORES):
        o = res.results[cidx]["out"].astype(np.float64)       # [2H, BL]
        pool = (2.0 / T) * (wo_f @ o[0:HID] + wo_b @ o[HID:])  # [3, BL]
        out_core = pool.T + fc_out_b                          # [BL, 3]
        outs.append(out_core)
    full = np.concatenate(outs, axis=0).astype(np.float32)
    if _want_trace:
        _CACHE["last_result"] = res
    return full


# revision 18
# speedup vs baseline: 1.2505x; 1.0005x over previous
"""Trainium2 Bass kernel for bidirectional ActionLSTM.

Full inputs in, full output out. Internally: data-parallel over batch
(8 NeuronCores x 256 batch rows), LSTM weights replicated.

Device program (per core, transposed layout: hidden on partitions,
batch on the free dim):
  - fc_in is folded into the LSTM input weights on the host:
        W_x = w_ih @ fc_in_w  [512, 68],  bias = w_ih@fc_in_b + b_ih + b_hh
    and the bias is folded in as an extra all-ones input row (K=69).
  - Gate order permuted to [i, f, o, g]. tanh is computed via the
    identity tanh(z) = 2*sigmoid(2z) - 1, with the 2z folded into the
    g-gate weight rows, so ONE sigmoid activation op covers all gates.
  - h is stored as h/2 ("h_half"); the 2x is folded into the recurrent
    and pooling weights. tanh(c) likewise becomes sigmoid(2c) via the
    activation's free scale.
  - Mean-pool over time + fc_out are folded into a per-step accumulating
    matmul into PSUM: pacc += (2/T * Wo_dir) @ h_half_t.
Per (step, dir): 9 matmuls (4 x-side K=69 + 4 recurrent K=128 + 1 pool),
1 big sigmoid [128,1024], 1 sigmoid(2c) [128,256] on ACT, 3
scalar_tensor_tensor ops on DVE, 1 tensor_tensor on GpSimd.
"""

import os
import numpy as np
from contextlib import ExitStack

INPUT, HID, NCLS = 68, 128, 3
B, T = 2048, 128
NCORES = 8
BL = B // NCORES          # 256 batch rows per core
KX = INPUT + 1            # 69 (ones row folds bias in)
G4 = 4 * HID              # 512

# matmul operand dtype: "f32r" (full fp32 storage, fast tensor-engine mode)
# or "bf16"
MM_DTYPE = os.environ.get("LSTM_MM_DTYPE", "bf16")
USE_FILLER = os.environ.get("LSTM_FILLER", "1") == "1"
N_FILLER = int(os.environ.get("LSTM_N_FILLER", "3"))
CELL_BF16 = os.environ.get("LSTM_CELL_BF16", "1") == "1"

_CACHE = {}


def _build_program():
    import concourse.bass as bass
    import concourse.tile as tile
    from concourse import bacc, mybir

    f32 = mybir.dt.float32
    AF = mybir.ActivationFunctionType
    OP = mybir.AluOpType
    use_bf16 = MM_DTYPE == "bf16"
    # dtype for tensors consumed by the tensor engine (x, weights, h):
    # bf16 runs the matmul at full rate (fp32r measured at 1/4 rate on HW);
    # gate accumulation stays fp32 in PSUM, s/c stay fp32 on DVE/ACT.
    sb_dt = mybir.dt.bfloat16 if use_bf16 else mybir.dt.float32r

    cell_dt = mybir.dt.bfloat16 if CELL_BF16 else f32

    def R(ap):
        return ap

    nc = bacc.Bacc("TRN2", target_bir_lowering=False, debug=False,
                   num_devices=NCORES)

    xin = nc.dram_tensor("xin", [KX, T * BL], sb_dt, kind="ExternalInput").ap()
    wx = {}
    wu = {}
    for d in "fb":
        wx[d] = nc.dram_tensor(f"wx_{d}", [KX, G4], sb_dt,
                               kind="ExternalInput").ap()
        wu[d] = nc.dram_tensor(f"wu_{d}", [HID, G4], sb_dt,
                               kind="ExternalInput").ap()
    out = nc.dram_tensor("out", [2 * HID, BL], f32, kind="ExternalOutput").ap()

    with tile.TileContext(nc) as tc, ExitStack() as ctx:
        const = ctx.enter_context(tc.tile_pool(name="const", bufs=1))
        X = const.tile([KX, T * BL], sb_dt, tag="X")
        # split the big input DMA into chunks so it spreads across DMA
        # queues and so early timesteps unblock compute quickly; issue
        # from both ends since the bwd direction consumes t=T-1 first.
        NCHUNK = 16
        CW = T * BL // NCHUNK
        order = []
        for i in range(NCHUNK // 2):
            order += [NCHUNK - 1 - i, i]
        for ci in order:
            nc.sync.dma_start(X[:, ci * CW:(ci + 1) * CW],
                              xin[:, ci * CW:(ci + 1) * CW])

        WX = {}
        WU = {}
        for d in "fb":
            WX[d] = const.tile([KX, G4], sb_dt, tag=f"wx{d}", name=f"WX{d}")
            nc.sync.dma_start(WX[d][:], wx[d][:])
            WU[d] = const.tile([HID, G4], sb_dt, tag=f"wu{d}", name=f"WU{d}")
            nc.sync.dma_start(WU[d][:], wu[d][:])

        hpool = ctx.enter_context(tc.tile_pool(name="h", bufs=3))
        cpool = ctx.enter_context(tc.tile_pool(name="c", bufs=3))
        spool = ctx.enter_context(tc.tile_pool(name="s", bufs=2))
        scpool = ctx.enter_context(tc.tile_pool(name="sc", bufs=2))
        mpool = ctx.enter_context(tc.tile_pool(name="m1h", bufs=2))
        tpool = ctx.enter_context(tc.tile_pool(name="tt", bufs=2))
        gpsum = ctx.enter_context(tc.tile_pool(name="gates", bufs=1,
                                               space="PSUM"))

        h = {}
        c = {}
        hsum = {}
        gates = {}
        for d in "fb":
            h[d] = hpool.tile([HID, BL], sb_dt, tag=f"h{d}", name=f"h0{d}")
            nc.vector.memset(h[d][:].bitcast(f32) if not use_bf16 else h[d][:],
                             0.0)
            c[d] = cpool.tile([HID, BL], cell_dt, tag=f"c{d}", name=f"c0{d}")
            nc.vector.memset(c[d][:], 0.0)
            hsum[d] = const.tile([HID, BL], f32, tag=f"hs{d}", name=f"hsum{d}")
            nc.vector.memset(hsum[d][:], 0.0)

        def emit_x_window(t):
            # x-side window matmuls for steps (t, t+1), both dirs. Emitted at
            # the END of the previous step so they execute during the sigmoid/
            # DVE phase instead of head-of-line-blocking the h-gated U
            # matmuls in the in-order PE queue.
            for d in ("f", "b"):
                g = gpsum.tile([HID, 4, 2 * BL], f32, tag=f"g{d}",
                               name=f"g_{d}_{t}")
                gates[d] = g
                if d == "f":
                    xw = X[:, t * BL:(t + 2) * BL]
                else:
                    xw = X[:, (T - 2 - t) * BL:(T - t) * BL]
                for gi in range(4):
                    nc.tensor.matmul(g[:, gi, :],
                                     R(WX[d][:, gi * HID:(gi + 1) * HID]),
                                     R(xw), start=True, stop=False,
                                     skip_group_check=True)

        emit_x_window(0)
        for t in range(T):
            # phase 1: recurrent matmuls + big sigmoid, per dir — the two
            # dirs' sigmoids sit adjacent in the ACT FIFO so ACT works dir b
            # while dir f's DVE chain runs (anti-phase staggering)
            sd = {}
            half = {}
            for d in ("f", "b"):
                g = gates[d]
                hf = t % 2 if d == "f" else 1 - (t % 2)
                half[d] = hf
                cs = hf * BL
                for gi in range(4):
                    nc.tensor.matmul(g[:, gi, cs:cs + BL],
                                     R(WU[d][:, gi * HID:(gi + 1) * HID]),
                                     R(h[d][:]), start=False, stop=(t % 2 == 1),
                                     skip_group_check=True)
                s = spool.tile([HID, 4, BL], cell_dt, tag=f"s{d}", name=f"s{d}{t}")
                nc.scalar.activation(s[:], g[:, :, cs:cs + BL], AF.Sigmoid)
                sd[d] = s
            # phase 2: cell updates, stage-interleaved across dirs
            ttd = {}
            md = {}
            cn = {}
            scd = {}
            for d in ("f", "b"):
                s = sd[d]
                tt = tpool.tile([HID, BL], cell_dt, tag=f"tt{d}", name=f"tt{d}{t}")
                nc.vector.tensor_tensor(tt[:], s[:, 1, :], c[d][:], op=OP.mult)
                m1h = mpool.tile([HID, BL], cell_dt, tag=f"m{d}", name=f"m{d}{t}")
                nc.vector.scalar_tensor_tensor(m1h[:], s[:, 3, :], 0.5,
                                               s[:, 0, :],
                                               op0=OP.subtract, op1=OP.mult)
                ttd[d] = tt
                md[d] = m1h
            for d in ("f", "b"):
                c_new = cpool.tile([HID, BL], cell_dt, tag=f"c{d}", name=f"c{d}{t}")
                nc.vector.tensor_tensor(c_new[:], md[d][:], ttd[d][:],
                                        op=OP.add)
                sc = scpool.tile([HID, BL], cell_dt, tag=f"sc{d}", name=f"sc{d}{t}")
                nc.scalar.activation(sc[:], c_new[:], AF.Sigmoid, scale=4.0)
                cn[d] = c_new
                scd[d] = sc
            for d in ("f", "b"):
                h_new = hpool.tile([HID, BL], sb_dt, tag=f"h{d}",
                                   name=f"h{d}{t}")
                nc.vector.scalar_tensor_tensor(h_new[:], scd[d][:], 0.5,
                                               sd[d][:, 2, :],
                                               op0=OP.subtract, op1=OP.mult)
                # pooling accumulate on the otherwise-idle GpSimd (off the
                # critical path, so the slow Q7 path is fine)
                nc.gpsimd.tensor_tensor(hsum[d][:], hsum[d][:], h_new[:],
                                        op=OP.add)
                h[d] = h_new
                c[d] = cn[d]
            if t % 2 == 1 and t < T - 1:
                emit_x_window(t + 1)

        nc.sync.dma_start(out[0:HID, :], hsum["f"][:])
        nc.sync.dma_start(out[HID:2 * HID, :], hsum["b"][:])

    nc.compile()
    return nc


def _prep_weights(w_ih, w_hh, b_ih, b_hh, fc_in_w, fc_in_b):
    Wx = w_ih.astype(np.float64) @ fc_in_w.astype(np.float64)   # [512, 68]
    bias = w_ih.astype(np.float64) @ fc_in_b.astype(np.float64) \
        + b_ih.astype(np.float64) + b_hh.astype(np.float64)
    perm = np.concatenate([np.arange(0, 128), np.arange(128, 256),
                           np.arange(384, 512), np.arange(256, 384)])
    Wx = Wx[perm]
    U = w_hh.astype(np.float64)[perm]
    bias = bias[perm]
    srow = np.ones((512, 1), np.float64)
    srow[384:] = 2.0
    Wx_aug = np.concatenate([Wx, bias[:, None]], axis=1)        # [512, 69]
    lhsT_x = np.ascontiguousarray((srow * Wx_aug).T)            # [69, 512]
    lhsT_U = np.ascontiguousarray((srow * U * 2.0).T)           # [128, 512]
    return lhsT_x, lhsT_U


def kernel(x, fc_in_w, fc_in_b, w_ih_f, w_hh_f, b_ih_f, b_hh_f,
           w_ih_b, w_hh_b, b_ih_b, b_hh_b, fc_out_w, fc_out_b,
           _want_trace=False):
    from concourse import bass_utils

    np_dt = np.float32
    if MM_DTYPE == "bf16":
        import ml_dtypes
        np_dt = ml_dtypes.bfloat16

    if "nc" not in _CACHE:
        _CACHE["nc"] = _build_program()
    nc = _CACHE["nc"]

    lx_f, lU_f = _prep_weights(w_ih_f, w_hh_f, b_ih_f, b_hh_f,
                               fc_in_w, fc_in_b)
    lx_b, lU_b = _prep_weights(w_ih_b, w_hh_b, b_ih_b, b_hh_b,
                               fc_in_w, fc_in_b)
    shared = {
        "wx_f": lx_f.astype(np_dt), "wu_f": lU_f.astype(np_dt),
        "wx_b": lx_b.astype(np_dt), "wu_b": lU_b.astype(np_dt),
    }
    wo_f = fc_out_w[:, :HID].astype(np.float64)   # [3, 128]
    wo_b = fc_out_w[:, HID:].astype(np.float64)
    in_maps = []
    for cidx in range(NCORES):
        xs = x[cidx * BL:(cidx + 1) * BL]                    # [BL, T, 68]
        xT = np.ascontiguousarray(xs.transpose(2, 1, 0))     # [68, T, BL]
        x_aug = np.concatenate(
            [xT, np.ones((1, T, BL), np.float32)], axis=0)   # [69, T, BL]
        in_maps.append({"xin": x_aug.reshape(KX, T * BL).astype(np_dt),
                        **shared})

    res = bass_utils.run_bass_kernel_spmd(
        nc, in_maps, core_ids=list(range(NCORES)), trace=_want_trace)
    outs = []
    for cidx in range(NCORES):
        o = res.results[cidx]["out"].astype(np.float64)       # [2H, BL]
        pool = (2.0 / T) * (wo_f @ o[0:HID] + wo_b @ o[HID:])  # [3, BL]
        out_core = pool.T + fc_out_b                          # [BL, 3]
        outs.append(out_core)
    full = np.concatenate(outs, axis=0).astype(np.float32)
    if _want_trace:
        _CACHE["last_result"] = res
    return full
